# revision 25
# baseline (speedup 1.0000x reference)
"""TransformerXL attention (AttentionXL) Bass kernel for Trainium2, 8 NeuronCores.

Sharding: pure data-parallel over batch (BS=8 -> 1 batch element per core).
All weights replicated per core; no collectives.

v6 design (fully fused pipeline; v5 was ~373us, v1 baseline ~389us):
  - exp factoring: A = exp((C+S)*s) = exp(C*s) * exp(S*s).  The position
    scores are exponentiated on the way out of PSUM, the rel-shift DMA
    gathers exp(S*s), and the combine is one DVE scalar_tensor_tensor
    (A = eC*eS with Z accumulated in the same instruction).  The causal
    mask becomes a multiplicative 0-fill on the diagonal 128-block.
  - Two fused phases with SBUF-lifetime-aware pools:
      phase 0: QT (kc-outer pairs) -> per-RT-chunk P-score production
               (matmul + expP + DRAM write).  RT/QvT/wq/wpos/xcT/pT die
               here, freeing SBUF+PSUM for phase 1.
      phase 1: per chunk ch: KT[ch+1] (one block ahead), C+softmax for
               heads 2ch/2ch+1, V slice, AV for the lagged head pair.
    Engine loads balance: scalar = expP (ph0) / expC (ph1), DVE =
    combine+norm, PE never waits on a same-block producer.
  - A^T via ONE DMA XBAR transpose per head ([128 i, (ib j)] ->
    [j%128, (ib jc), i%128]); AV reads 3-level [part, ib, u] APs.
    a_t rotates over 3 tiles so AV lags the transpose by a full head.
  - P matmuls/exps trimmed to the m-range the rel-shift reads
    (m >= 384-128*ib); persistent pa/a_sb tiles with one-time memsets
    keep every byte under the DMAs initialized (race-detector clean).
  - Per-matrix per-chunk input DMAs ordered wq,xc -> wpos,pT -> wk,wv,xT
    so the first QT matmul starts after ~3MB instead of 13.6MB.
  - Final projection bias via broadcast b_out tile + DVE add fused into
    the PSUM drain; per-i-block output DMAs.

Per-core algorithm (bf16 on the PE, fp32 PSUM accumulation):
  Host prep:  X^T, Xc^T, Pos^T, W_kv split into W_k/W_v, bias folds:
                bias_qu = b_q + u.ravel(), bias_qv = b_q + v.ravel()
                b_out   = b_v @ W_proj + b_proj  (softmax rows sum to 1)
  Device:
    KT = W_k^T @ X^T   [hd, j]   (+b_k)      RT = W_pos^T @ P^T [hd, m]
    QT = W_q^T @ Xc^T  [hd, i]  -> QuT/QvT   V  = X^T.T @ W_v   [j, hd]
    per head h:
      eP [i,m] = exp(QvT_h^T RT_h * s) -> DRAM
      eS [i,j] = ePflat[i*1023 + 511 + j]  (rel-shift gather), diag 0-mask
      eC [i,j] = exp(QuT_h^T KT_h * s)  (ScalarE from PSUM)
      A = eC*eS, Z = sum_j A  (DVE);  A *= 1/Z
      A^T via DMA XBAR transpose -> a_t [j%128, (ib,jc), i%128]
      O^T_h [d, i] = V_h^T A^T (PE, col-packed head pairs) -> AVT
    out[i,e] = AVT^T @ W_proj (+ b_out via DVE broadcast add), fp32.
"""

import os
import sys

for _p in (
    "/root/.axon_site",
    "/root/.axon_site/_ro/trn_rl_repo",
    "/root/.axon_site/_ro/pypackages",
    "/opt/trn_rl_repo",
):
    if os.path.isdir(_p) and _p not in sys.path:
        sys.path.append(_p)

import numpy as np
import ml_dtypes

import concourse.bass as bass
import concourse.mybir as mybir
import concourse.tile as tile
from concourse.bass_utils import run_bass_kernel_spmd

BF16 = mybir.dt.bfloat16
FP32 = mybir.dt.float32
AF = mybir.ActivationFunctionType
ALU = mybir.AluOpType
nbf16 = ml_dtypes.bfloat16

CUR, FULL, BS, DIM, H, D = 512, 1024, 8, 1024, 16, 64
PREV = FULL - CUR
SCALE = 1.0 / D**0.5
P = 128
NIB = CUR // P    # 4 query blocks
NJC = FULL // P   # 8 key chunks
NCH = DIM // P    # 8 dim chunks
NHP = H // 2      # 8 head pairs

_BUILT = None


def _mlo(ib):
    # lowest m the rel-shift gather reads within i-block ib
    return max(0, 384 - 128 * ib)


def _split_multiwait(nc):
    """walrus here encodes at most ONE sync wait per TPB instruction
    (NEURON_ISA_TPB_EVENTS has a single wait slot).  Split every
    multi-wait instruction: prepend same-engine NoOps carrying the
    extra waits, keep the last wait on the instruction itself."""
    n_split = 0
    for fn in nc.m.functions:
        for blk in fn.blocks:
            insts = list(blk.instructions)
            out = []
            for ins in insts:
                si = ins.sync_info
                if si is not None and si.on_wait and len(si.on_wait) > 1:
                    waits = list(si.on_wait)
                    for w in waits[:-1]:
                        nop = mybir.InstNoOp(
                            name=f"{ins.name}-ws{n_split}",
                            engine=ins.engine,
                            sync_info=mybir.SyncInfo(on_wait=[w], on_update=[]),
                            text_hint="waitsplit",
                        )
                        out.append(nop)
                        n_split += 1
                    ins.sync_info = mybir.SyncInfo(
                        on_wait=[waits[-1]],
                        on_update=list(si.on_update or []),
                    )
                out.append(ins)
            blk.instructions = out
    return n_split


def _build(split_waits=True):
    nc = bass.Bass()

    # acts: [X^T | Xc^T | Pos^T] cols; wmats: [W_q | W_pos | W_k | W_v] cols
    acts = nc.declare_dram_parameter("acts", [DIM, FULL + CUR + FULL], BF16, isOutput=False)
    wmats = nc.declare_dram_parameter("wmats", [DIM, 4 * DIM], BF16, isOutput=False)
    wproj = nc.declare_dram_parameter("wproj", [DIM, DIM], BF16, isOutput=False)
    # biases pre-laid-out on host: [p, 4*NCH] = qu | qv | k | pos chunks
    biases = nc.declare_dram_parameter("biases", [P, 4 * NCH], FP32, isOutput=False)
    bout = nc.declare_dram_parameter("bout", [DIM], BF16, isOutput=False)
    out = nc.declare_dram_parameter("out", [CUR, DIM], FP32, isOutput=True)

    with tile.TileContext(nc) as tc:
        from contextlib import ExitStack

        with ExitStack() as ctx:
            persist = ctx.enter_context(tc.tile_pool(name="persist", bufs=1))

            KT = persist.tile([P, NCH, FULL], BF16, tag="KT")
            V = persist.tile([P, NJC, DIM], BF16, tag="V")
            QuT = persist.tile([P, NCH, CUR], BF16, tag="QuT")
            AVT = persist.tile([P, NCH, CUR], BF16, tag="AVT")
            bout_b = persist.tile([P, DIM], BF16, tag="bout_b")
            bias_t = persist.tile([P, 4, NCH], FP32, tag="bias_t")  # qu|qv|k|pos

            mask_zero_reg = nc.gpsimd.to_reg(0.0)
            nc.sync.dma_start(bias_t, biases.rearrange("p (b c) -> p b c", b=4))
            nc.sync.dma_start(
                bout_b, bass.AP(tensor=bout, offset=0, ap=[[0, P], [1, DIM]])
            )

            pdram_tiles = [None] * H
            sexp_tiles = [None] * H
            dram = ctx.enter_context(tc.tile_pool(name="dram", bufs=16, space="DRAM"))
            # whole-kernel inputs: xT, wk, wv
            a2 = ctx.enter_context(tc.tile_pool(name="a2", bufs=1))
            xT_t = a2.tile([P, NCH, FULL], BF16, tag="xT")
            wk_t = a2.tile([P, NCH, DIM], BF16, tag="wk")
            wv_t = a2.tile([P, NCH, DIM], BF16, tag="wv")

            acts_r = acts.rearrange("(c p) f -> p c f", p=P)
            wmats_r = wmats.rearrange("(c p) f -> p c f", p=P)

            # ---------- phase 0: QT, RT + exp(P)-score production ----------
            with tc.tile_pool(name="a1", bufs=1) as a1, tc.tile_pool(
                name="pps", bufs=3, space="PSUM"
            ) as pps:
                RT = a1.tile([P, NCH, FULL], BF16, tag="RT")
                QvT = a1.tile([P, NCH, CUR], BF16, tag="QvT")
                pa = [a1.tile([P, NIB, FULL], BF16, tag="pa0", name="pa0"),
                      a1.tile([P, NIB, FULL], BF16, tag="pa1", name="pa1")]
                xcT_t = a1.tile([P, NCH, CUR], BF16, tag="xcT")
                pT_t = a1.tile([P, NCH, FULL], BF16, tag="pT")
                wq_t = a1.tile([P, NCH, DIM], BF16, tag="wq")
                wpos_t = a1.tile([P, NCH, DIM], BF16, tag="wpos")

                # input loads, consumer-ordered: (wq,xc) -> (wpos,pT) -> rest
                for c in range(NCH):
                    nc.sync.dma_start(wq_t[:, c : c + 1], wmats_r[:, c : c + 1, 0:DIM])
                    nc.sync.dma_start(
                        xcT_t[:, c : c + 1], acts_r[:, c : c + 1, FULL : FULL + CUR]
                    )
                for c in range(NCH):
                    nc.sync.dma_start(
                        wpos_t[:, c : c + 1], wmats_r[:, c : c + 1, DIM : 2 * DIM]
                    )
                    nc.sync.dma_start(
                        pT_t[:, c : c + 1], acts_r[:, c : c + 1, FULL + CUR :]
                    )
                for c in range(NCH):
                    nc.sync.dma_start(
                        wk_t[:, c : c + 1], wmats_r[:, c : c + 1, 2 * DIM : 3 * DIM]
                    )
                    nc.sync.dma_start(
                        wv_t[:, c : c + 1], wmats_r[:, c : c + 1, 3 * DIM : 4 * DIM]
                    )
                    nc.sync.dma_start(xT_t[:, c : c + 1], acts_r[:, c : c + 1, 0:FULL])

                for t in pa:
                    for ib in range(NIB - 1):
                        nc.vector.memset(t[:, ib, 0 : _mlo(ib)], 0.0)

                # Q^T [hd, i]: kc-outer pairs so matmuls consume chunks as
                # they land instead of waiting for the full load.
                for grp in range(4):
                    ocs = range(grp * 2, grp * 2 + 2)
                    pq = pps.tile([P, FULL], FP32, tag="pp", name=f"qg{grp}")
                    pss = {oc: pq[:, (oc % 2) * CUR : (oc % 2 + 1) * CUR]
                           for oc in ocs}
                    for kc in range(NCH):
                        for oc in ocs:
                            nc.tensor.matmul(
                                pss[oc],
                                wq_t[:, kc, oc * P : (oc + 1) * P],
                                xcT_t[:, kc, :],
                                start=(kc == 0),
                                stop=(kc == NCH - 1),
                            )
                    for oc in ocs:
                        nc.scalar.activation(
                            QuT[:, oc, :], pss[oc], AF.Identity,
                            bias=bias_t[:, 0, oc : oc + 1],
                        )
                        nc.vector.tensor_scalar_add(
                            QvT[:, oc, :], pss[oc], bias_t[:, 1, oc : oc + 1]
                        )

                def emit_P(h):
                    """exp(P*s) production for head h: matmuls + expP + DRAM."""
                    ch, ro = divmod(h, 2)
                    ro *= D
                    rs = slice(ro, ro + D)
                    p_all = pa[h % 2]
                    for ib in range(NIB):
                        isl = slice(ib * P, (ib + 1) * P)
                        mlo = _mlo(ib)
                        pp = pps.tile([P, FULL], FP32, tag="pp", name="pp")
                        nc.tensor.matmul(
                            pp[:, mlo:512], QvT[rs, ch, isl], RT[rs, ch, mlo:512],
                            start=True, stop=True,
                        )
                        nc.tensor.matmul(
                            pp[:, 512:FULL], QvT[rs, ch, isl], RT[rs, ch, 512:FULL],
                            start=True, stop=True,
                        )
                        nc.scalar.activation(
                            p_all[:, ib, mlo:], pp[:, mlo:], AF.Exp, scale=SCALE
                        )
                    pdram = dram.tile([CUR, FULL], BF16, tag="pdram", name="pdram")
                    nc.sync.dma_start(
                        pdram.rearrange("(ib p) m -> p ib m", p=P), p_all
                    )
                    pdram_tiles[h] = pdram

                # R^T chunks, each followed by the P production it unblocks
                for ch in range(NCH):
                    pr = pps.tile([P, FULL], FP32, tag="pp", name="pr")
                    for jh in range(2):
                        sl = slice(jh * 512, (jh + 1) * 512)
                        ps = pr[:, sl]
                        for kc in range(NCH):
                            nc.tensor.matmul(
                                ps,
                                wpos_t[:, kc, ch * P : (ch + 1) * P],
                                pT_t[:, kc, sl],
                                start=(kc == 0),
                                stop=(kc == NCH - 1),
                            )
                        if jh == 0:
                            nc.scalar.activation(
                                RT[:, ch, sl], ps, AF.Identity,
                                bias=bias_t[:, 3, ch : ch + 1],
                            )
                        else:
                            nc.vector.tensor_scalar_add(
                                RT[:, ch, sl], ps, bias_t[:, 3, ch : ch + 1]
                            )
                    emit_P(2 * ch)
                    emit_P(2 * ch + 1)
                    vs = {3: [(0, 0), (0, 1), (0, 2)],
                          4: [(0, 3), (0, 4), (0, 5)],
                          5: [(0, 6), (0, 7), (1, 0)],
                          6: [(1, 1), (1, 2), (1, 3)],
                          7: [(1, 4), (1, 5), (1, 6), (1, 7)]}.get(ch, [])
                    for mh, jc in vs:
                        sl = slice(mh * 512, (mh + 1) * 512)
                        pv = pps.tile([P, FULL], FP32, tag="pp", name="pv")
                        ps = pv[:, 0:512]
                        for kc in range(NCH):
                            nc.tensor.matmul(
                                ps,
                                xT_t[:, kc, jc * P : (jc + 1) * P],
                                wv_t[:, kc, sl],
                                start=(kc == 0),
                                stop=(kc == NCH - 1),
                            )
                        nc.vector.tensor_copy(V[:, jc, sl], ps)

            # ---------- phase 1: KT + C/softmax + V + AV, fused ----------
            late = ctx.enter_context(tc.tile_pool(name="late", bufs=1))
            sall = ctx.enter_context(tc.tile_pool(name="sall", bufs=4))
            work = ctx.enter_context(tc.tile_pool(name="work", bufs=4))
            cps = ctx.enter_context(tc.tile_pool(name="cps", bufs=3, space="PSUM"))
            avp = ctx.enter_context(tc.tile_pool(name="avp", bufs=1, space="PSUM"))

            WPROJ = late.tile([P, NCH, DIM], BF16, tag="WPROJ")
            nc.sync.dma_start(WPROJ, wproj.rearrange("(c p) f -> p c f", p=P))
            # packed A staging, 4-deep rotation: per-ib slice [off, off+jmax)
            # (expC(h) then only waits on XBAR(h-3), 1.5 blocks back)
            ASB_OFF = [0, 640, 1408, 2304]
            ASB_W = 3328
            asb = [late.tile([P, ASB_W], BF16, tag=f"as{k}", name=f"as{k}")
                   for k in range(3)]
            # a_t rotation depth 4: AV(hp) runs a block after both its
            # heads' transposes, and no XBAR before it can touch their tiles
            atl = [late.tile([P, NIB, NJC, P], BF16, tag=f"at{k}", name=f"at{k}")
                   for k in range(4)]

            def emit_gather(h):
                """rel-shift gather of exp(S*s) + multiplicative diag mask."""
                pdram = pdram_tiles[h]
                s_exp = sall.tile([P, NIB, FULL], BF16, tag="s_exp", name="s_exp")
                sh_ap = bass.AP(
                    tensor=pdram.tensor,
                    offset=pdram.offset + (PREV - 1),
                    ap=[[FULL - 1, P], [(FULL - 1) * P, NIB], [1, FULL]],
                )
                nc.sync.dma_start(s_exp, sh_ap)
                # causal mask: zero the over-diagonal in the diagonal block
                # (keep iff u - j' >= 0; u = i%128, j' = j-512-128*ib)
                for ib in range(NIB):
                    j0 = 512 + ib * P
                    nc.gpsimd.affine_select(
                        out=s_exp[:, ib, j0 : j0 + P],
                        in_=s_exp[:, ib, j0 : j0 + P],
                        compare_op=ALU.is_ge,
                        fill=mask_zero_reg,
                        base=0,
                        channel_multiplier=1,
                        pattern=[[-1, P]],
                    )
                sexp_tiles[h] = s_exp
                pdram_tiles[h] = None

            def emit_KT(ch):
                for jh in range(2):
                    sl = slice(jh * 512, (jh + 1) * 512)
                    ps = avp.tile([P, CUR], FP32, tag=("av_a", "av_b")[jh],
                                  name="kps")
                    for kc in range(NCH):
                        nc.tensor.matmul(
                            ps,
                            wk_t[:, kc, ch * P : (ch + 1) * P],
                            xT_t[:, kc, sl],
                            start=(kc == 0),
                            stop=(kc == NCH - 1),
                        )
                    nc.scalar.activation(
                        KT[:, ch, sl], ps, AF.Identity,
                        bias=bias_t[:, 2, ch : ch + 1],
                    )

            def emit_softmax(h):
                ch, ro = divmod(h, 2)
                ro *= D
                rs = slice(ro, ro + D)
                a_sb = asb[h % 3]
                s_exp = sexp_tiles[h]
                for ib in range(NIB):
                    isl = slice(ib * P, (ib + 1) * P)
                    jmax = 640 + ib * P
                    asl = slice(ASB_OFF[ib], ASB_OFF[ib] + jmax)
                    cp = cps.tile([P, FULL], FP32, tag="cp")
                    nc.tensor.matmul(
                        cp[:, 0:512], QuT[rs, ch, isl], KT[rs, ch, 0:512],
                        start=True, stop=True,
                    )
                    nc.tensor.matmul(
                        cp[:, 512:jmax], QuT[rs, ch, isl], KT[rs, ch, 512:jmax],
                        start=True, stop=True,
                    )
                    nc.scalar.activation(
                        a_sb[:, asl], cp[:, :jmax], AF.Exp, scale=SCALE
                    )
                    z_t = work.tile([P, 1], FP32, tag="z_t")
                    # A = eC * eS with Z = sum_j A fused
                    nc.vector.scalar_tensor_tensor(
                        out=a_sb[:, asl],
                        in0=a_sb[:, asl],
                        scalar=1.0,
                        in1=s_exp[:, ib, :jmax],
                        op0=ALU.mult,
                        op1=ALU.mult,
                        accum_out=z_t,
                    )
                    rz = work.tile([P, 1], FP32, tag="rz")
                    nc.vector.reciprocal(rz, z_t)
                    nc.vector.tensor_scalar_mul(a_sb[:, asl], a_sb[:, asl], rz)
                    # per-i-block XBAR: [128 i, jmax j] -> [j%128, jc, i%128]
                    nc.sync.dma_start(
                        atl[h % 4][:, ib, 0 : jmax // P], a_sb[:, asl],
                        transpose=True,
                    )
                sexp_tiles[h] = None
                return atl[h % 4]

            at_of = [None] * H

            def emit_AV(hp):
                av2 = [avp.tile([P, CUR], FP32, tag="av_a", name="av_a"),
                       avp.tile([P, CUR], FP32, tag="av_b", name="av_b")]
                for jc in range(NJC):
                    ibmin = max(0, jc - 4)
                    for hh in range(2):
                        hx = 2 * hp + hh
                        nc.tensor.matmul(
                            av2[hh][hh * D : (hh + 1) * D, ibmin * P :],
                            V[:, jc, hx * D : (hx + 1) * D],
                            at_of[hx][:, ibmin:, jc, :],
                            start=(jc == 0),
                            stop=(jc == NJC - 1),
                            tile_position=(0, hh * D),
                        )
                nc.vector.tensor_copy(AVT[0:D, hp, :], av2[0][0:D, :])
                nc.vector.tensor_copy(AVT[D:P, hp, :], av2[1][D:P, :])

            emit_KT(0)
            emit_gather(0)
            emit_gather(1)
            for ch in range(NCH):
                # gathers first: their data is ready, so they issue instantly;
                # XBARs behind them may wait without starving anything near
                if 2 * ch + 2 < H:
                    emit_gather(2 * ch + 2)
                if 2 * ch + 3 < H:
                    emit_gather(2 * ch + 3)
                if ch + 1 < NCH:
                    emit_KT(ch + 1)
                at_of[2 * ch] = emit_softmax(2 * ch)
                at_of[2 * ch + 1] = emit_softmax(2 * ch + 1)
                if ch >= 1:
                    emit_AV(ch - 1)
            emit_AV(NHP - 1)

            # ---------------- Final projection ----------------
            with tc.tile_pool(name="fin", bufs=2) as fin:
                out_r = out.rearrange("(ib p) e -> p ib e", p=P)
                for ib in range(NIB):
                    isl = slice(ib * P, (ib + 1) * P)
                    o_ib = fin.tile([P, DIM], FP32, tag="o_ib", name="o_ib")
                    for eh in range(2):
                        esl = slice(eh * 512, (eh + 1) * 512)
                        fp = avp.tile([P, 512], FP32, tag=("av_a", "av_b")[eh],
                                      name="fp")
                        for fc in range(NCH):
                            nc.tensor.matmul(
                                fp, AVT[:, fc, isl], WPROJ[:, fc, esl],
                                start=(fc == 0), stop=(fc == NCH - 1),
                            )
                        nc.vector.tensor_tensor(
                            o_ib[:, esl], fp, bout_b[:, esl], ALU.add
                        )
                    nc.sync.dma_start(out_r[:, ib, :], o_ib)

    if split_waits:
        _split_multiwait(nc)
    return nc


def _get_nc():
    global _BUILT
    if _BUILT is None:
        _BUILT = _build()
    return _BUILT


def _prep_host(inputs, pos_embedding, full_input, u, v, mask,
               W_kv, b_kv, W_q, b_q, W_pos, b_pos, W_proj, b_proj):
    f32 = np.float32
    W_k = np.ascontiguousarray(W_kv[:, : H * D])
    W_v = np.ascontiguousarray(W_kv[:, H * D :])
    b_k = b_kv[: H * D].astype(f32)
    b_v = b_kv[H * D :].astype(f32)
    bias_qu = (b_q + u.ravel()).astype(f32)
    bias_qv = (b_q + v.ravel()).astype(f32)
    b_out = (b_v @ W_proj + b_proj).astype(f32)

    bias_all = np.stack(
        [bias_qu.reshape(NCH, P), bias_qv.reshape(NCH, P),
         b_k.reshape(NCH, P), b_pos.astype(f32).reshape(NCH, P)], axis=0
    )  # [4, NCH, P]
    bias_all = np.ascontiguousarray(bias_all.transpose(2, 0, 1).reshape(P, 4 * NCH))
    wmats_np = np.concatenate([W_q, W_pos, W_k, W_v], axis=1).astype(nbf16)
    shared = {
        "wmats": wmats_np,
        "wproj": W_proj.astype(nbf16),
        "biases": bias_all.astype(f32),
        "bout": b_out.astype(nbf16),
    }
    pT_np = pos_embedding[:, 0].T
    in_maps = []
    for c in range(BS):
        m = dict(shared)
        m["acts"] = np.concatenate(
            [full_input[:, c].T, inputs[:, c].T, pT_np], axis=1
        ).astype(nbf16)
        in_maps.append(m)
    return in_maps


def kernel(**inputs):
    nc = _get_nc()
    in_maps = _prep_host(**{k: np.asarray(v) for k, v in inputs.items()})
    res = run_bass_kernel_spmd(nc, in_maps, list(range(BS)))
    out = np.stack([res.results[c]["out"] for c in range(BS)], axis=1)
    return np.ascontiguousarray(out.astype(np.float32))


if __name__ == "__main__":
    nc = _build()
    print("built ok")


# revision 26
# speedup vs baseline: 1.1483x; 1.1483x over previous
"""TransformerXL attention (AttentionXL) Bass kernel for Trainium2, 8 NeuronCores.

Sharding: pure data-parallel over batch (BS=8 -> 1 batch element per core).
All weights replicated per core; no collectives.

v6 design (fully fused pipeline; v5 was ~373us, v1 baseline ~389us):
  - exp factoring: A = exp((C+S)*s) = exp(C*s) * exp(S*s).  The position
    scores are exponentiated on the way out of PSUM, the rel-shift DMA
    gathers exp(S*s), and the combine is one DVE scalar_tensor_tensor
    (A = eC*eS with Z accumulated in the same instruction).  The causal
    mask becomes a multiplicative 0-fill on the diagonal 128-block.
  - Two fused phases with SBUF-lifetime-aware pools:
      phase 0: QT (kc-outer pairs) -> per-RT-chunk P-score production
               (matmul + expP + DRAM write).  RT/QvT/wq/wpos/xcT/pT die
               here, freeing SBUF+PSUM for phase 1.
      phase 1: per chunk ch: KT[ch+1] (one block ahead), C+softmax for
               heads 2ch/2ch+1, V slice, AV for the lagged head pair.
    Engine loads balance: scalar = expP (ph0) / expC (ph1), DVE =
    combine+norm, PE never waits on a same-block producer.
  - A^T via ONE DMA XBAR transpose per head ([128 i, (ib j)] ->
    [j%128, (ib jc), i%128]); AV reads 3-level [part, ib, u] APs.
    a_t rotates over 3 tiles so AV lags the transpose by a full head.
  - P matmuls/exps trimmed to the m-range the rel-shift reads
    (m >= 384-128*ib); persistent pa/a_sb tiles with one-time memsets
    keep every byte under the DMAs initialized (race-detector clean).
  - Per-matrix per-chunk input DMAs ordered wq,xc -> wpos,pT -> wk,wv,xT
    so the first QT matmul starts after ~3MB instead of 13.6MB.
  - Final projection bias via broadcast b_out tile + DVE add fused into
    the PSUM drain; per-i-block output DMAs.

Per-core algorithm (bf16 on the PE, fp32 PSUM accumulation):
  Host prep:  X^T, Xc^T, Pos^T, W_kv split into W_k/W_v, bias folds:
                bias_qu = b_q + u.ravel(), bias_qv = b_q + v.ravel()
                b_out   = b_v @ W_proj + b_proj  (softmax rows sum to 1)
  Device:
    KT = W_k^T @ X^T   [hd, j]   (+b_k)      RT = W_pos^T @ P^T [hd, m]
    QT = W_q^T @ Xc^T  [hd, i]  -> QuT/QvT   V  = X^T.T @ W_v   [j, hd]
    per head h:
      eP [i,m] = exp(QvT_h^T RT_h * s) -> DRAM
      eS [i,j] = ePflat[i*1023 + 511 + j]  (rel-shift gather), diag 0-mask
      eC [i,j] = exp(QuT_h^T KT_h * s)  (ScalarE from PSUM)
      A = eC*eS, Z = sum_j A  (DVE);  A *= 1/Z
      A^T via DMA XBAR transpose -> a_t [j%128, (ib,jc), i%128]
      O^T_h [d, i] = V_h^T A^T (PE, col-packed head pairs) -> AVT
    out[i,e] = AVT^T @ W_proj (+ b_out via DVE broadcast add), fp32.
"""

import os
import sys

for _p in (
    "/root/.axon_site",
    "/root/.axon_site/_ro/trn_rl_repo",
    "/root/.axon_site/_ro/pypackages",
    "/opt/trn_rl_repo",
):
    if os.path.isdir(_p) and _p not in sys.path:
        sys.path.append(_p)

import numpy as np
import ml_dtypes

import concourse.bass as bass
import concourse.mybir as mybir
import concourse.tile as tile
from concourse.bass_utils import run_bass_kernel_spmd

BF16 = mybir.dt.bfloat16
FP32 = mybir.dt.float32
AF = mybir.ActivationFunctionType
ALU = mybir.AluOpType
nbf16 = ml_dtypes.bfloat16

CUR, FULL, BS, DIM, H, D = 512, 1024, 8, 1024, 16, 64
PREV = FULL - CUR
SCALE = 1.0 / D**0.5
P = 128
NIB = CUR // P    # 4 query blocks
NJC = FULL // P   # 8 key chunks
NCH = DIM // P    # 8 dim chunks
NHP = H // 2      # 8 head pairs

_BUILT = None


def _mlo(ib):
    # lowest m the rel-shift gather reads within i-block ib
    return max(0, 384 - 128 * ib)


def _split_multiwait(nc):
    """walrus here encodes at most ONE sync wait per TPB instruction
    (NEURON_ISA_TPB_EVENTS has a single wait slot).  Split every
    multi-wait instruction: prepend same-engine NoOps carrying the
    extra waits, keep the last wait on the instruction itself."""
    n_split = 0
    for fn in nc.m.functions:
        for blk in fn.blocks:
            insts = list(blk.instructions)
            out = []
            for ins in insts:
                si = ins.sync_info
                if si is not None and si.on_wait and len(si.on_wait) > 1:
                    waits = list(si.on_wait)
                    for w in waits[:-1]:
                        nop = mybir.InstNoOp(
                            name=f"{ins.name}-ws{n_split}",
                            engine=ins.engine,
                            sync_info=mybir.SyncInfo(on_wait=[w], on_update=[]),
                            text_hint="waitsplit",
                        )
                        out.append(nop)
                        n_split += 1
                    ins.sync_info = mybir.SyncInfo(
                        on_wait=[waits[-1]],
                        on_update=list(si.on_update or []),
                    )
                out.append(ins)
            blk.instructions = out
    return n_split


def _build(split_waits=True):
    nc = bass.Bass()

    # acts: [X^T | Xc^T | Pos^T] cols; wmats: [W_q | W_pos | W_k | W_v] cols
    acts = nc.declare_dram_parameter("acts", [DIM, FULL + CUR + FULL], BF16, isOutput=False)
    wmats = nc.declare_dram_parameter("wmats", [DIM, 4 * DIM], BF16, isOutput=False)
    wproj = nc.declare_dram_parameter("wproj", [DIM, DIM], BF16, isOutput=False)
    # biases pre-laid-out on host: [p, 4*NCH] = qu | qv | k | pos chunks
    biases = nc.declare_dram_parameter("biases", [P, 4 * NCH], FP32, isOutput=False)
    bout = nc.declare_dram_parameter("bout", [DIM], BF16, isOutput=False)
    out = nc.declare_dram_parameter("out", [CUR, DIM], FP32, isOutput=True)

    with tile.TileContext(nc) as tc:
        from contextlib import ExitStack

        with ExitStack() as ctx:
            persist = ctx.enter_context(tc.tile_pool(name="persist", bufs=1))

            KT = persist.tile([P, NCH, FULL], BF16, tag="KT")
            V = persist.tile([P, NJC, DIM], BF16, tag="V")
            QuT = persist.tile([P, NCH, CUR], BF16, tag="QuT")
            AVT = persist.tile([P, NCH, CUR], BF16, tag="AVT")
            bout_b = persist.tile([P, DIM], BF16, tag="bout_b")
            bias_t = persist.tile([P, 4, NCH], FP32, tag="bias_t")  # qu|qv|k|pos

            mask_zero_reg = nc.gpsimd.to_reg(0.0)
            nc.sync.dma_start(bias_t, biases.rearrange("p (b c) -> p b c", b=4))
            nc.sync.dma_start(
                bout_b, bass.AP(tensor=bout, offset=0, ap=[[0, P], [1, DIM]])
            )

            pdram_tiles = [None] * H
            sexp_tiles = [None] * H
            dram = ctx.enter_context(tc.tile_pool(name="dram", bufs=16, space="DRAM"))
            # whole-kernel inputs: xT, wk, wv
            a2 = ctx.enter_context(tc.tile_pool(name="a2", bufs=1))
            xT_t = a2.tile([P, NCH, FULL], BF16, tag="xT")
            wk_t = a2.tile([P, NCH, DIM], BF16, tag="wk")
            wv_t = a2.tile([P, NCH, DIM], BF16, tag="wv")
            apsum = ctx.enter_context(tc.tile_pool(name="apsum", bufs=2, space="PSUM"))

            acts_r = acts.rearrange("(c p) f -> p c f", p=P)
            wmats_r = wmats.rearrange("(c p) f -> p c f", p=P)

            # ---------- phase 0: QT, RT + exp(P)-score production ----------
            with tc.tile_pool(name="a1", bufs=1) as a1, tc.tile_pool(
                name="pps", bufs=2, space="PSUM"
            ) as pps:
                RT = a1.tile([P, NCH, FULL], BF16, tag="RT")
                QvT = a1.tile([P, NCH, CUR], BF16, tag="QvT")
                pa = [a1.tile([P, NIB, FULL], BF16, tag="pa0", name="pa0"),
                      a1.tile([P, NIB, FULL], BF16, tag="pa1", name="pa1")]
                xcT_t = a1.tile([P, NCH, CUR], BF16, tag="xcT")
                pT_t = a1.tile([P, NCH, FULL], BF16, tag="pT")
                wq_t = a1.tile([P, NCH, DIM], BF16, tag="wq")
                wpos_t = a1.tile([P, NCH, DIM], BF16, tag="wpos")

                # input loads, consumer-ordered: (wq,xc) -> (wpos,pT) -> rest
                for c in range(NCH):
                    nc.sync.dma_start(wq_t[:, c : c + 1], wmats_r[:, c : c + 1, 0:DIM])
                    nc.sync.dma_start(
                        xcT_t[:, c : c + 1], acts_r[:, c : c + 1, FULL : FULL + CUR]
                    )
                for c in range(NCH):
                    nc.sync.dma_start(
                        wpos_t[:, c : c + 1], wmats_r[:, c : c + 1, DIM : 2 * DIM]
                    )
                    nc.sync.dma_start(
                        pT_t[:, c : c + 1], acts_r[:, c : c + 1, FULL + CUR :]
                    )
                for c in range(NCH):
                    nc.sync.dma_start(
                        wk_t[:, c : c + 1], wmats_r[:, c : c + 1, 2 * DIM : 3 * DIM]
                    )
                    nc.sync.dma_start(
                        wv_t[:, c : c + 1], wmats_r[:, c : c + 1, 3 * DIM : 4 * DIM]
                    )
                    nc.sync.dma_start(xT_t[:, c : c + 1], acts_r[:, c : c + 1, 0:FULL])

                for t in pa:
                    for ib in range(NIB - 1):
                        nc.vector.memset(t[:, ib, 0 : _mlo(ib)], 0.0)

                # Q^T [hd, i]: kc-outer pairs so matmuls consume chunks as
                # they land instead of waiting for the full load.
                for grp in range(4):
                    ocs = range(grp * 2, grp * 2 + 2)
                    pss = {
                        oc: apsum.tile([P, CUR], FP32, tag="aps", name=f"qps{oc}")
                        for oc in ocs
                    }
                    for kc in range(NCH):
                        for oc in ocs:
                            nc.tensor.matmul(
                                pss[oc],
                                wq_t[:, kc, oc * P : (oc + 1) * P],
                                xcT_t[:, kc, :],
                                start=(kc == 0),
                                stop=(kc == NCH - 1),
                            )
                    for oc in ocs:
                        nc.scalar.activation(
                            QuT[:, oc, :], pss[oc], AF.Identity,
                            bias=bias_t[:, 0, oc : oc + 1],
                        )
                        nc.vector.tensor_scalar_add(
                            QvT[:, oc, :], pss[oc], bias_t[:, 1, oc : oc + 1]
                        )

                def emit_P(h):
                    """exp(P*s) production for head h: matmuls + expP + DRAM."""
                    ch, ro = divmod(h, 2)
                    ro *= D
                    rs = slice(ro, ro + D)
                    p_all = pa[h % 2]
                    for ib in range(NIB):
                        isl = slice(ib * P, (ib + 1) * P)
                        mlo = _mlo(ib)
                        pp = pps.tile([P, FULL], FP32, tag="pp", name="pp")
                        nc.tensor.matmul(
                            pp[:, mlo:512], QvT[rs, ch, isl], RT[rs, ch, mlo:512],
                            start=True, stop=True,
                        )
                        nc.tensor.matmul(
                            pp[:, 512:FULL], QvT[rs, ch, isl], RT[rs, ch, 512:FULL],
                            start=True, stop=True,
                        )
                        nc.scalar.activation(
                            p_all[:, ib, mlo:], pp[:, mlo:], AF.Exp, scale=SCALE
                        )
                    pdram = dram.tile([CUR, FULL], BF16, tag="pdram", name="pdram")
                    nc.sync.dma_start(
                        pdram.rearrange("(ib p) m -> p ib m", p=P), p_all
                    )
                    pdram_tiles[h] = pdram

                # R^T chunks, each followed by the P production it unblocks
                for ch in range(NCH):
                    for jh in range(2):
                        sl = slice(jh * 512, (jh + 1) * 512)
                        ps = apsum.tile([P, 512], FP32, tag="aps")
                        for kc in range(NCH):
                            nc.tensor.matmul(
                                ps,
                                wpos_t[:, kc, ch * P : (ch + 1) * P],
                                pT_t[:, kc, sl],
                                start=(kc == 0),
                                stop=(kc == NCH - 1),
                            )
                        if jh == 0:
                            nc.scalar.activation(
                                RT[:, ch, sl], ps, AF.Identity,
                                bias=bias_t[:, 3, ch : ch + 1],
                            )
                        else:
                            nc.vector.tensor_scalar_add(
                                RT[:, ch, sl], ps, bias_t[:, 3, ch : ch + 1]
                            )
                    emit_P(2 * ch)
                    emit_P(2 * ch + 1)
                    vs = {3: [(0, 0), (0, 1), (0, 2)],
                          4: [(0, 3), (0, 4), (0, 5)],
                          5: [(0, 6), (0, 7), (1, 0)],
                          6: [(1, 1), (1, 2), (1, 3)],
                          7: [(1, 4), (1, 5), (1, 6), (1, 7)]}.get(ch, [])
                    for mh, jc in vs:
                        sl = slice(mh * 512, (mh + 1) * 512)
                        ps = apsum.tile([P, 512], FP32, tag="aps")
                        for kc in range(NCH):
                            nc.tensor.matmul(
                                ps,
                                xT_t[:, kc, jc * P : (jc + 1) * P],
                                wv_t[:, kc, sl],
                                start=(kc == 0),
                                stop=(kc == NCH - 1),
                            )
                        nc.vector.tensor_copy(V[:, jc, sl], ps)

            # ---------- phase 1: KT + C/softmax + V + AV, fused ----------
            late = ctx.enter_context(tc.tile_pool(name="late", bufs=1))
            sall = ctx.enter_context(tc.tile_pool(name="sall", bufs=4))
            work = ctx.enter_context(tc.tile_pool(name="work", bufs=4))
            cps = ctx.enter_context(tc.tile_pool(name="cps", bufs=2, space="PSUM"))
            avp = ctx.enter_context(tc.tile_pool(name="avp", bufs=1, space="PSUM"))

            WPROJ = late.tile([P, NCH, DIM], BF16, tag="WPROJ")
            nc.sync.dma_start(WPROJ, wproj.rearrange("(c p) f -> p c f", p=P))
            # packed A staging, 4-deep rotation: per-ib slice [off, off+jmax)
            # (expC(h) then only waits on XBAR(h-3), 1.5 blocks back)
            ASB_OFF = [0, 640, 1408, 2304]
            ASB_W = 3328
            asb = [late.tile([P, ASB_W], BF16, tag=f"as{k}", name=f"as{k}")
                   for k in range(3)]
            # a_t rotation depth 4: AV(hp) runs a block after both its
            # heads' transposes, and no XBAR before it can touch their tiles
            atl = [late.tile([P, NIB, NJC, P], BF16, tag=f"at{k}", name=f"at{k}")
                   for k in range(4)]

            def emit_gather(h):
                """rel-shift gather of exp(S*s) + multiplicative diag mask."""
                pdram = pdram_tiles[h]
                s_exp = sall.tile([P, NIB, FULL], BF16, tag="s_exp", name="s_exp")
                sh_ap = bass.AP(
                    tensor=pdram.tensor,
                    offset=pdram.offset + (PREV - 1),
                    ap=[[FULL - 1, P], [(FULL - 1) * P, NIB], [1, FULL]],
                )
                nc.sync.dma_start(s_exp, sh_ap)
                # causal mask: zero the over-diagonal in the diagonal block
                # (keep iff u - j' >= 0; u = i%128, j' = j-512-128*ib)
                for ib in range(NIB):
                    j0 = 512 + ib * P
                    nc.gpsimd.affine_select(
                        out=s_exp[:, ib, j0 : j0 + P],
                        in_=s_exp[:, ib, j0 : j0 + P],
                        compare_op=ALU.is_ge,
                        fill=mask_zero_reg,
                        base=0,
                        channel_multiplier=1,
                        pattern=[[-1, P]],
                    )
                sexp_tiles[h] = s_exp
                pdram_tiles[h] = None

            def emit_KT(ch):
                for jh in range(2):
                    sl = slice(jh * 512, (jh + 1) * 512)
                    ps = apsum.tile([P, 512], FP32, tag="aps")
                    for kc in range(NCH):
                        nc.tensor.matmul(
                            ps,
                            wk_t[:, kc, ch * P : (ch + 1) * P],
                            xT_t[:, kc, sl],
                            start=(kc == 0),
                            stop=(kc == NCH - 1),
                        )
                    nc.scalar.activation(
                        KT[:, ch, sl], ps, AF.Identity,
                        bias=bias_t[:, 2, ch : ch + 1],
                    )

            def emit_softmax(h):
                ch, ro = divmod(h, 2)
                ro *= D
                rs = slice(ro, ro + D)
                a_sb = asb[h % 3]
                s_exp = sexp_tiles[h]
                for ib in range(NIB):
                    isl = slice(ib * P, (ib + 1) * P)
                    jmax = 640 + ib * P
                    asl = slice(ASB_OFF[ib], ASB_OFF[ib] + jmax)
                    cp = cps.tile([P, FULL], FP32, tag="cp")
                    nc.tensor.matmul(
                        cp[:, 0:512], QuT[rs, ch, isl], KT[rs, ch, 0:512],
                        start=True, stop=True,
                    )
                    nc.tensor.matmul(
                        cp[:, 512:jmax], QuT[rs, ch, isl], KT[rs, ch, 512:jmax],
                        start=True, stop=True,
                    )
                    nc.scalar.activation(
                        a_sb[:, asl], cp[:, :jmax], AF.Exp, scale=SCALE
                    )
                    z_t = work.tile([P, 1], FP32, tag="z_t")
                    # A = eC * eS with Z = sum_j A fused
                    nc.vector.scalar_tensor_tensor(
                        out=a_sb[:, asl],
                        in0=a_sb[:, asl],
                        scalar=1.0,
                        in1=s_exp[:, ib, :jmax],
                        op0=ALU.mult,
                        op1=ALU.mult,
                        accum_out=z_t,
                    )
                    rz = work.tile([P, 1], FP32, tag="rz")
                    nc.vector.reciprocal(rz, z_t)
                    nc.vector.tensor_scalar_mul(a_sb[:, asl], a_sb[:, asl], rz)
                    # per-i-block XBAR: [128 i, jmax j] -> [j%128, jc, i%128]
                    nc.sync.dma_start(
                        atl[h % 4][:, ib, 0 : jmax // P], a_sb[:, asl],
                        transpose=True,
                    )
                sexp_tiles[h] = None
                return atl[h % 4]

            at_of = [None] * H

            def emit_AV(hp):
                av2 = [avp.tile([P, CUR], FP32, tag="av_a", name="av_a"),
                       avp.tile([P, CUR], FP32, tag="av_b", name="av_b")]
                for jc in range(NJC):
                    ibmin = max(0, jc - 4)
                    for hh in range(2):
                        hx = 2 * hp + hh
                        nc.tensor.matmul(
                            av2[hh][hh * D : (hh + 1) * D, ibmin * P :],
                            V[:, jc, hx * D : (hx + 1) * D],
                            at_of[hx][:, ibmin:, jc, :],
                            start=(jc == 0),
                            stop=(jc == NJC - 1),
                            tile_position=(0, hh * D),
                        )
                nc.vector.tensor_copy(AVT[0:D, hp, :], av2[0][0:D, :])
                nc.vector.tensor_copy(AVT[D:P, hp, :], av2[1][D:P, :])

            emit_KT(0)
            emit_gather(0)
            emit_gather(1)
            for ch in range(NCH):
                # gathers first: their data is ready, so they issue instantly;
                # XBARs behind them may wait without starving anything near
                if 2 * ch + 2 < H:
                    emit_gather(2 * ch + 2)
                if 2 * ch + 3 < H:
                    emit_gather(2 * ch + 3)
                if ch + 1 < NCH:
                    emit_KT(ch + 1)
                at_of[2 * ch] = emit_softmax(2 * ch)
                at_of[2 * ch + 1] = emit_softmax(2 * ch + 1)
                if ch >= 1:
                    emit_AV(ch - 1)
            emit_AV(NHP - 1)

            # ---------------- Final projection ----------------
            with tc.tile_pool(name="fin", bufs=2) as fin:
                out_r = out.rearrange("(ib p) e -> p ib e", p=P)
                for ib in range(NIB):
                    isl = slice(ib * P, (ib + 1) * P)
                    o_ib = fin.tile([P, DIM], FP32, tag="o_ib", name="o_ib")
                    for eh in range(2):
                        esl = slice(eh * 512, (eh + 1) * 512)
                        fp = avp.tile([P, 512], FP32, tag=("av_a", "av_b")[eh],
                                      name="fp")
                        for fc in range(NCH):
                            nc.tensor.matmul(
                                fp, AVT[:, fc, isl], WPROJ[:, fc, esl],
                                start=(fc == 0), stop=(fc == NCH - 1),
                            )
                        nc.vector.tensor_tensor(
                            o_ib[:, esl], fp, bout_b[:, esl], ALU.add
                        )
                    nc.sync.dma_start(out_r[:, ib, :], o_ib)

    if split_waits:
        _split_multiwait(nc)
    return nc


def _get_nc():
    global _BUILT
    if _BUILT is None:
        _BUILT = _build()
    return _BUILT


def _prep_host(inputs, pos_embedding, full_input, u, v, mask,
               W_kv, b_kv, W_q, b_q, W_pos, b_pos, W_proj, b_proj):
    f32 = np.float32
    W_k = np.ascontiguousarray(W_kv[:, : H * D])
    W_v = np.ascontiguousarray(W_kv[:, H * D :])
    b_k = b_kv[: H * D].astype(f32)
    b_v = b_kv[H * D :].astype(f32)
    bias_qu = (b_q + u.ravel()).astype(f32)
    bias_qv = (b_q + v.ravel()).astype(f32)
    b_out = (b_v @ W_proj + b_proj).astype(f32)

    bias_all = np.stack(
        [bias_qu.reshape(NCH, P), bias_qv.reshape(NCH, P),
         b_k.reshape(NCH, P), b_pos.astype(f32).reshape(NCH, P)], axis=0
    )  # [4, NCH, P]
    bias_all = np.ascontiguousarray(bias_all.transpose(2, 0, 1).reshape(P, 4 * NCH))
    wmats_np = np.concatenate([W_q, W_pos, W_k, W_v], axis=1).astype(nbf16)
    shared = {
        "wmats": wmats_np,
        "wproj": W_proj.astype(nbf16),
        "biases": bias_all.astype(f32),
        "bout": b_out.astype(nbf16),
    }
    pT_np = pos_embedding[:, 0].T
    in_maps = []
    for c in range(BS):
        m = dict(shared)
        m["acts"] = np.concatenate(
            [full_input[:, c].T, inputs[:, c].T, pT_np], axis=1
        ).astype(nbf16)
        in_maps.append(m)
    return in_maps


def kernel(**inputs):
    nc = _get_nc()
    in_maps = _prep_host(**{k: np.asarray(v) for k, v in inputs.items()})
    res = run_bass_kernel_spmd(nc, in_maps, list(range(BS)))
    out = np.stack([res.results[c]["out"] for c in range(BS)], axis=1)
    return np.ascontiguousarray(out.astype(np.float32))


if __name__ == "__main__":
    nc = _build()
    print("built ok")


# revision 29
# speedup vs baseline: 1.1618x; 1.0118x over previous
"""TransformerXL attention (AttentionXL) Bass kernel for Trainium2, 8 NeuronCores.

Sharding: pure data-parallel over batch (BS=8 -> 1 batch element per core).
All weights replicated per core; no collectives.

v6 design (fully fused pipeline; v5 was ~373us, v1 baseline ~389us):
  - exp factoring: A = exp((C+S)*s) = exp(C*s) * exp(S*s).  The position
    scores are exponentiated on the way out of PSUM, the rel-shift DMA
    gathers exp(S*s), and the combine is one DVE scalar_tensor_tensor
    (A = eC*eS with Z accumulated in the same instruction).  The causal
    mask becomes a multiplicative 0-fill on the diagonal 128-block.
  - Two fused phases with SBUF-lifetime-aware pools:
      phase 0: QT (kc-outer pairs) -> per-RT-chunk P-score production
               (matmul + expP + DRAM write).  RT/QvT/wq/wpos/xcT/pT die
               here, freeing SBUF+PSUM for phase 1.
      phase 1: per chunk ch: KT[ch+1] (one block ahead), C+softmax for
               heads 2ch/2ch+1, V slice, AV for the lagged head pair.
    Engine loads balance: scalar = expP (ph0) / expC (ph1), DVE =
    combine+norm, PE never waits on a same-block producer.
  - A^T via ONE DMA XBAR transpose per head ([128 i, (ib j)] ->
    [j%128, (ib jc), i%128]); AV reads 3-level [part, ib, u] APs.
    a_t rotates over 3 tiles so AV lags the transpose by a full head.
  - P matmuls/exps trimmed to the m-range the rel-shift reads
    (m >= 384-128*ib); persistent pa/a_sb tiles with one-time memsets
    keep every byte under the DMAs initialized (race-detector clean).
  - Per-matrix per-chunk input DMAs ordered wq,xc -> wpos,pT -> wk,wv,xT
    so the first QT matmul starts after ~3MB instead of 13.6MB.
  - Final projection bias via broadcast b_out tile + DVE add fused into
    the PSUM drain; per-i-block output DMAs.

Per-core algorithm (bf16 on the PE, fp32 PSUM accumulation):
  Host prep:  X^T, Xc^T, Pos^T, W_kv split into W_k/W_v, bias folds:
                bias_qu = b_q + u.ravel(), bias_qv = b_q + v.ravel()
                b_out   = b_v @ W_proj + b_proj  (softmax rows sum to 1)
  Device:
    KT = W_k^T @ X^T   [hd, j]   (+b_k)      RT = W_pos^T @ P^T [hd, m]
    QT = W_q^T @ Xc^T  [hd, i]  -> QuT/QvT   V  = X^T.T @ W_v   [j, hd]
    per head h:
      eP [i,m] = exp(QvT_h^T RT_h * s) -> DRAM
      eS [i,j] = ePflat[i*1023 + 511 + j]  (rel-shift gather), diag 0-mask
      eC [i,j] = exp(QuT_h^T KT_h * s)  (ScalarE from PSUM)
      A = eC*eS, Z = sum_j A  (DVE);  A *= 1/Z
      A^T via DMA XBAR transpose -> a_t [j%128, (ib,jc), i%128]
      O^T_h [d, i] = V_h^T A^T (PE, col-packed head pairs) -> AVT
    out[i,e] = AVT^T @ W_proj (+ b_out via DVE broadcast add), fp32.
"""

import os
import sys

for _p in (
    "/root/.axon_site",
    "/root/.axon_site/_ro/trn_rl_repo",
    "/root/.axon_site/_ro/pypackages",
    "/opt/trn_rl_repo",
):
    if os.path.isdir(_p) and _p not in sys.path:
        sys.path.append(_p)

import numpy as np
import ml_dtypes

import concourse.bass as bass
import concourse.mybir as mybir
import concourse.tile as tile
from concourse.bass_utils import run_bass_kernel_spmd

BF16 = mybir.dt.bfloat16
FP32 = mybir.dt.float32
AF = mybir.ActivationFunctionType
ALU = mybir.AluOpType
nbf16 = ml_dtypes.bfloat16

CUR, FULL, BS, DIM, H, D = 512, 1024, 8, 1024, 16, 64
PREV = FULL - CUR
SCALE = 1.0 / D**0.5
P = 128
NIB = CUR // P    # 4 query blocks
NJC = FULL // P   # 8 key chunks
NCH = DIM // P    # 8 dim chunks
NHP = H // 2      # 8 head pairs

_BUILT = None


def _mlo(ib):
    # lowest m the rel-shift gather reads within i-block ib
    return max(0, 384 - 128 * ib)


def _split_multiwait(nc):
    """walrus here encodes at most ONE sync wait per TPB instruction
    (NEURON_ISA_TPB_EVENTS has a single wait slot).  Split every
    multi-wait instruction: prepend same-engine NoOps carrying the
    extra waits, keep the last wait on the instruction itself."""
    n_split = 0
    for fn in nc.m.functions:
        for blk in fn.blocks:
            insts = list(blk.instructions)
            out = []
            for ins in insts:
                si = ins.sync_info
                if si is not None and si.on_wait and len(si.on_wait) > 1:
                    waits = list(si.on_wait)
                    for w in waits[:-1]:
                        nop = mybir.InstNoOp(
                            name=f"{ins.name}-ws{n_split}",
                            engine=ins.engine,
                            sync_info=mybir.SyncInfo(on_wait=[w], on_update=[]),
                            text_hint="waitsplit",
                        )
                        out.append(nop)
                        n_split += 1
                    ins.sync_info = mybir.SyncInfo(
                        on_wait=[waits[-1]],
                        on_update=list(si.on_update or []),
                    )
                out.append(ins)
            blk.instructions = out
    return n_split


def _build(split_waits=True):
    nc = bass.Bass()

    # acts: [X^T | Xc^T | Pos^T] cols; wmats: [W_q | W_pos | W_k | W_v] cols
    acts = nc.declare_dram_parameter("acts", [DIM, FULL + CUR + FULL], BF16, isOutput=False)
    wmats = nc.declare_dram_parameter("wmats", [DIM, 4 * DIM], BF16, isOutput=False)
    wproj = nc.declare_dram_parameter("wproj", [DIM, DIM], BF16, isOutput=False)
    # biases pre-laid-out on host: [p, 4*NCH] = qu | qv | k | pos chunks
    biases = nc.declare_dram_parameter("biases", [P, 4 * NCH], FP32, isOutput=False)
    bout = nc.declare_dram_parameter("bout", [DIM], BF16, isOutput=False)
    out = nc.declare_dram_parameter("out", [CUR, DIM], FP32, isOutput=True)

    with tile.TileContext(nc) as tc:
        from contextlib import ExitStack

        with ExitStack() as ctx:
            persist = ctx.enter_context(tc.tile_pool(name="persist", bufs=1))

            KT = persist.tile([P, NCH, FULL], BF16, tag="KT")
            V = persist.tile([P, NJC, DIM], BF16, tag="V")
            QuT = persist.tile([P, NCH, CUR], BF16, tag="QuT")
            AVT = persist.tile([P, NCH, CUR], BF16, tag="AVT")
            bout_b = persist.tile([P, DIM], BF16, tag="bout_b")
            bias_t = persist.tile([P, 4, NCH], FP32, tag="bias_t")  # qu|qv|k|pos

            mask_zero_reg = nc.gpsimd.to_reg(0.0)
            nc.sync.dma_start(bias_t, biases.rearrange("p (b c) -> p b c", b=4))
            nc.sync.dma_start(
                bout_b, bass.AP(tensor=bout, offset=0, ap=[[0, P], [1, DIM]])
            )

            pdram_tiles = [None] * H
            sexp_tiles = [None] * H
            dram = ctx.enter_context(tc.tile_pool(name="dram", bufs=16, space="DRAM"))
            # whole-kernel inputs: xT, wk, wv
            a2 = ctx.enter_context(tc.tile_pool(name="a2", bufs=1))
            xT_t = a2.tile([P, NCH, FULL], BF16, tag="xT")
            wk_t = a2.tile([P, NCH, DIM], BF16, tag="wk")
            wv_t = a2.tile([P, NCH, DIM], BF16, tag="wv")
            apsum = ctx.enter_context(tc.tile_pool(name="apsum", bufs=2, space="PSUM"))

            acts_r = acts.rearrange("(c p) f -> p c f", p=P)
            wmats_r = wmats.rearrange("(c p) f -> p c f", p=P)

            # ---------- phase 0: QT, RT + exp(P)-score production ----------
            with tc.tile_pool(name="a1", bufs=1) as a1, tc.tile_pool(
                name="pps", bufs=2, space="PSUM"
            ) as pps:
                RT = a1.tile([P, NCH, FULL], BF16, tag="RT")
                QvT = a1.tile([P, NCH, CUR], BF16, tag="QvT")
                pa = [a1.tile([P, NIB, FULL], BF16, tag="pa0", name="pa0"),
                      a1.tile([P, NIB, FULL], BF16, tag="pa1", name="pa1")]
                xcT_t = a1.tile([P, NCH, CUR], BF16, tag="xcT")
                pT_t = a1.tile([P, NCH, FULL], BF16, tag="pT")
                wq_t = a1.tile([P, NCH, DIM], BF16, tag="wq")
                wpos_t = a1.tile([P, NCH, DIM], BF16, tag="wpos")

                # input loads, consumer-ordered: (wq,xc) -> (wpos,pT) -> rest
                for c in range(NCH):
                    nc.sync.dma_start(wq_t[:, c : c + 1], wmats_r[:, c : c + 1, 0:DIM])
                    nc.sync.dma_start(
                        xcT_t[:, c : c + 1], acts_r[:, c : c + 1, FULL : FULL + CUR]
                    )
                for c in range(NCH):
                    nc.sync.dma_start(
                        wpos_t[:, c : c + 1], wmats_r[:, c : c + 1, DIM : 2 * DIM]
                    )
                    nc.sync.dma_start(
                        pT_t[:, c : c + 1], acts_r[:, c : c + 1, FULL + CUR :]
                    )
                for c in range(NCH):
                    nc.sync.dma_start(
                        wk_t[:, c : c + 1], wmats_r[:, c : c + 1, 2 * DIM : 3 * DIM]
                    )
                    nc.sync.dma_start(
                        wv_t[:, c : c + 1], wmats_r[:, c : c + 1, 3 * DIM : 4 * DIM]
                    )
                    nc.sync.dma_start(xT_t[:, c : c + 1], acts_r[:, c : c + 1, 0:FULL])

                for t in pa:
                    for ib in range(NIB - 1):
                        nc.vector.memset(t[:, ib, 0 : _mlo(ib)], 0.0)

                # Q^T [hd, i]: kc-outer pairs so matmuls consume chunks as
                # they land instead of waiting for the full load.
                for grp in range(4):
                    ocs = range(grp * 2, grp * 2 + 2)
                    pss = {
                        oc: apsum.tile([P, CUR], FP32, tag="aps", name=f"qps{oc}")
                        for oc in ocs
                    }
                    for kc in range(NCH):
                        for oc in ocs:
                            nc.tensor.matmul(
                                pss[oc],
                                wq_t[:, kc, oc * P : (oc + 1) * P],
                                xcT_t[:, kc, :],
                                start=(kc == 0),
                                stop=(kc == NCH - 1),
                            )
                    for oc in ocs:
                        nc.scalar.activation(
                            QuT[:, oc, :], pss[oc], AF.Identity,
                            bias=bias_t[:, 0, oc : oc + 1],
                        )
                        nc.vector.tensor_scalar_add(
                            QvT[:, oc, :], pss[oc], bias_t[:, 1, oc : oc + 1]
                        )

                def emit_P(h):
                    """exp(P*s) production for head h: matmuls + expP + DRAM."""
                    ch, ro = divmod(h, 2)
                    ro *= D
                    rs = slice(ro, ro + D)
                    p_all = pa[h % 2]
                    for ib in range(NIB):
                        isl = slice(ib * P, (ib + 1) * P)
                        mlo = _mlo(ib)
                        pp = pps.tile([P, FULL], FP32, tag="pp", name="pp")
                        nc.tensor.matmul(
                            pp[:, mlo:512], QvT[rs, ch, isl], RT[rs, ch, mlo:512],
                            start=True, stop=True,
                        )
                        nc.tensor.matmul(
                            pp[:, 512:FULL], QvT[rs, ch, isl], RT[rs, ch, 512:FULL],
                            start=True, stop=True,
                        )
                        nc.scalar.activation(
                            p_all[:, ib, mlo:], pp[:, mlo:], AF.Exp, scale=SCALE
                        )
                    pdram = dram.tile([CUR, FULL], BF16, tag="pdram", name="pdram")
                    nc.sync.dma_start(
                        pdram.rearrange("(ib p) m -> p ib m", p=P), p_all
                    )
                    pdram_tiles[h] = pdram

                # R^T chunks, each followed by the P production it unblocks
                for ch in range(NCH):
                    for jh in range(2):
                        sl = slice(jh * 512, (jh + 1) * 512)
                        ps = apsum.tile([P, 512], FP32, tag="aps")
                        for kc in range(NCH):
                            nc.tensor.matmul(
                                ps,
                                wpos_t[:, kc, ch * P : (ch + 1) * P],
                                pT_t[:, kc, sl],
                                start=(kc == 0),
                                stop=(kc == NCH - 1),
                            )
                        if jh == 0:
                            nc.scalar.activation(
                                RT[:, ch, sl], ps, AF.Identity,
                                bias=bias_t[:, 3, ch : ch + 1],
                            )
                        else:
                            nc.vector.tensor_scalar_add(
                                RT[:, ch, sl], ps, bias_t[:, 3, ch : ch + 1]
                            )
                    emit_P(2 * ch)
                    emit_P(2 * ch + 1)
                    vs = {3: [(0, 0), (0, 1), (0, 2)],
                          4: [(0, 3), (0, 4), (0, 5)],
                          5: [(0, 6), (0, 7), (1, 0)],
                          6: [(1, 1), (1, 2), (1, 3)],
                          7: [(1, 4), (1, 5), (1, 6), (1, 7)]}.get(ch, [])
                    for mh, jc in vs:
                        sl = slice(mh * 512, (mh + 1) * 512)
                        ps = apsum.tile([P, 512], FP32, tag="aps")
                        for kc in range(NCH):
                            nc.tensor.matmul(
                                ps,
                                xT_t[:, kc, jc * P : (jc + 1) * P],
                                wv_t[:, kc, sl],
                                start=(kc == 0),
                                stop=(kc == NCH - 1),
                            )
                        nc.vector.tensor_copy(V[:, jc, sl], ps)

            # ---------- phase 1: KT + C/softmax + V + AV, fused ----------
            late = ctx.enter_context(tc.tile_pool(name="late", bufs=1))
            sall = ctx.enter_context(tc.tile_pool(name="sall", bufs=4))
            work = ctx.enter_context(tc.tile_pool(name="work", bufs=4))
            cps = ctx.enter_context(tc.tile_pool(name="cps", bufs=2, space="PSUM"))
            avp = ctx.enter_context(tc.tile_pool(name="avp", bufs=1, space="PSUM"))

            WPROJ = late.tile([P, NCH, DIM], BF16, tag="WPROJ")
            nc.sync.dma_start(WPROJ, wproj.rearrange("(c p) f -> p c f", p=P))
            # packed A staging, 4-deep rotation: per-ib slice [off, off+jmax)
            # (expC(h) then only waits on XBAR(h-3), 1.5 blocks back)
            ASB_OFF = [0, 640, 1408, 2304]
            ASB_W = 3328
            asb = [late.tile([P, ASB_W], BF16, tag=f"as{k}", name=f"as{k}")
                   for k in range(3)]
            # a_t rotation depth 4: AV(hp) runs a block after both its
            # heads' transposes, and no XBAR before it can touch their tiles
            atl = [late.tile([P, NIB, NJC, P], BF16, tag=f"at{k}", name=f"at{k}")
                   for k in range(4)]

            def emit_gather(h):
                """rel-shift gather of exp(S*s) + multiplicative diag mask."""
                pdram = pdram_tiles[h]
                s_exp = sall.tile([P, NIB, FULL], BF16, tag="s_exp", name="s_exp")
                sh_ap = bass.AP(
                    tensor=pdram.tensor,
                    offset=pdram.offset + (PREV - 1),
                    ap=[[FULL - 1, P], [(FULL - 1) * P, NIB], [1, FULL]],
                )
                nc.sync.dma_start(s_exp, sh_ap)
                # causal mask: zero the over-diagonal in the diagonal block
                # (keep iff u - j' >= 0; u = i%128, j' = j-512-128*ib)
                for ib in range(NIB):
                    j0 = 512 + ib * P
                    nc.gpsimd.affine_select(
                        out=s_exp[:, ib, j0 : j0 + P],
                        in_=s_exp[:, ib, j0 : j0 + P],
                        compare_op=ALU.is_ge,
                        fill=mask_zero_reg,
                        base=0,
                        channel_multiplier=1,
                        pattern=[[-1, P]],
                    )
                sexp_tiles[h] = s_exp
                pdram_tiles[h] = None

            def emit_KT(ch):
                for jh in range(2):
                    sl = slice(jh * 512, (jh + 1) * 512)
                    ps = apsum.tile([P, 512], FP32, tag="aps")
                    for kc in range(NCH):
                        nc.tensor.matmul(
                            ps,
                            wk_t[:, kc, ch * P : (ch + 1) * P],
                            xT_t[:, kc, sl],
                            start=(kc == 0),
                            stop=(kc == NCH - 1),
                        )
                    nc.scalar.activation(
                        KT[:, ch, sl], ps, AF.Identity,
                        bias=bias_t[:, 2, ch : ch + 1],
                    )

            def emit_softmax(h):
                ch, ro = divmod(h, 2)
                ro *= D
                rs = slice(ro, ro + D)
                a_sb = asb[h % 3]
                s_exp = sexp_tiles[h]
                for ib in range(NIB):
                    isl = slice(ib * P, (ib + 1) * P)
                    jmax = 640 + ib * P
                    asl = slice(ASB_OFF[ib], ASB_OFF[ib] + jmax)
                    cp = cps.tile([P, FULL], FP32, tag="cp")
                    nc.tensor.matmul(
                        cp[:, 0:512], QuT[rs, ch, isl], KT[rs, ch, 0:512],
                        start=True, stop=True,
                    )
                    nc.tensor.matmul(
                        cp[:, 512:jmax], QuT[rs, ch, isl], KT[rs, ch, 512:jmax],
                        start=True, stop=True,
                    )
                    nc.scalar.activation(
                        a_sb[:, asl], cp[:, :jmax], AF.Exp, scale=SCALE
                    )
                    z_t = work.tile([P, 1], FP32, tag="z_t")
                    # A = eC * eS with Z = sum_j A fused
                    nc.vector.scalar_tensor_tensor(
                        out=a_sb[:, asl],
                        in0=a_sb[:, asl],
                        scalar=1.0,
                        in1=s_exp[:, ib, :jmax],
                        op0=ALU.mult,
                        op1=ALU.mult,
                        accum_out=z_t,
                    )
                    rz = work.tile([P, 1], FP32, tag="rz")
                    nc.vector.reciprocal(rz, z_t)
                    nc.vector.tensor_scalar_mul(a_sb[:, asl], a_sb[:, asl], rz)
                    # per-i-block XBAR: [128 i, jmax j] -> [j%128, jc, i%128]
                    nc.sync.dma_start(
                        atl[h % 4][:, ib, 0 : jmax // P], a_sb[:, asl],
                        transpose=True,
                    )
                sexp_tiles[h] = None
                return atl[h % 4]

            at_of = [None] * H

            def emit_AV(hp):
                av2 = [avp.tile([P, CUR], FP32, tag="av_a", name="av_a"),
                       avp.tile([P, CUR], FP32, tag="av_b", name="av_b")]
                for jc in range(NJC):
                    ibmin = max(0, jc - 4)
                    for hh in range(2):
                        hx = 2 * hp + hh
                        nc.tensor.matmul(
                            av2[hh][hh * D : (hh + 1) * D, ibmin * P :],
                            V[:, jc, hx * D : (hx + 1) * D],
                            at_of[hx][:, ibmin:, jc, :],
                            start=(jc == 0),
                            stop=(jc == NJC - 1),
                            tile_position=(0, hh * D),
                        )
                nc.vector.tensor_copy(AVT[0:D, hp, :], av2[0][0:D, :])
                nc.vector.tensor_copy(AVT[D:P, hp, :], av2[1][D:P, :])

            emit_KT(0)
            emit_gather(0)
            emit_gather(1)
            for ch in range(NCH):
                # gathers first: their data is ready, so they issue instantly;
                # XBARs behind them may wait without starving anything near
                if 2 * ch + 2 < H:
                    emit_gather(2 * ch + 2)
                if 2 * ch + 3 < H:
                    emit_gather(2 * ch + 3)
                # softmax first so the scalar expC segment starts at block
                # top (the cps WAR for the NEXT block clears ~3us earlier);
                # KT(ch+1) PE work then overlaps this block's softmax chain
                at_of[2 * ch] = emit_softmax(2 * ch)
                at_of[2 * ch + 1] = emit_softmax(2 * ch + 1)
                if ch + 1 < NCH:
                    emit_KT(ch + 1)
                if ch >= 1:
                    emit_AV(ch - 1)
            emit_AV(NHP - 1)

            # ---------------- Final projection ----------------
            with tc.tile_pool(name="fin", bufs=2) as fin:
                out_r = out.rearrange("(ib p) e -> p ib e", p=P)
                for ib in range(NIB):
                    isl = slice(ib * P, (ib + 1) * P)
                    o_ib = fin.tile([P, DIM], FP32, tag="o_ib", name="o_ib")
                    for eh in range(2):
                        esl = slice(eh * 512, (eh + 1) * 512)
                        fp = avp.tile([P, 512], FP32, tag=("av_a", "av_b")[eh],
                                      name="fp")
                        for fc in range(NCH):
                            nc.tensor.matmul(
                                fp, AVT[:, fc, isl], WPROJ[:, fc, esl],
                                start=(fc == 0), stop=(fc == NCH - 1),
                            )
                        nc.vector.tensor_tensor(
                            o_ib[:, esl], fp, bout_b[:, esl], ALU.add
                        )
                    nc.sync.dma_start(out_r[:, ib, :], o_ib)

    if split_waits:
        _split_multiwait(nc)
    return nc


def _get_nc():
    global _BUILT
    if _BUILT is None:
        _BUILT = _build()
    return _BUILT


def _prep_host(inputs, pos_embedding, full_input, u, v, mask,
               W_kv, b_kv, W_q, b_q, W_pos, b_pos, W_proj, b_proj):
    f32 = np.float32
    W_k = np.ascontiguousarray(W_kv[:, : H * D])
    W_v = np.ascontiguousarray(W_kv[:, H * D :])
    b_k = b_kv[: H * D].astype(f32)
    b_v = b_kv[H * D :].astype(f32)
    bias_qu = (b_q + u.ravel()).astype(f32)
    bias_qv = (b_q + v.ravel()).astype(f32)
    b_out = (b_v @ W_proj + b_proj).astype(f32)

    bias_all = np.stack(
        [bias_qu.reshape(NCH, P), bias_qv.reshape(NCH, P),
         b_k.reshape(NCH, P), b_pos.astype(f32).reshape(NCH, P)], axis=0
    )  # [4, NCH, P]
    bias_all = np.ascontiguousarray(bias_all.transpose(2, 0, 1).reshape(P, 4 * NCH))
    wmats_np = np.concatenate([W_q, W_pos, W_k, W_v], axis=1).astype(nbf16)
    shared = {
        "wmats": wmats_np,
        "wproj": W_proj.astype(nbf16),
        "biases": bias_all.astype(f32),
        "bout": b_out.astype(nbf16),
    }
    pT_np = pos_embedding[:, 0].T
    in_maps = []
    for c in range(BS):
        m = dict(shared)
        m["acts"] = np.concatenate(
            [full_input[:, c].T, inputs[:, c].T, pT_np], axis=1
        ).astype(nbf16)
        in_maps.append(m)
    return in_maps


def kernel(**inputs):
    nc = _get_nc()
    in_maps = _prep_host(**{k: np.asarray(v) for k, v in inputs.items()})
    res = run_bass_kernel_spmd(nc, in_maps, list(range(BS)))
    out = np.stack([res.results[c]["out"] for c in range(BS)], axis=1)
    return np.ascontiguousarray(out.astype(np.float32))


if __name__ == "__main__":
    nc = _build()
    print("built ok")
